# revision 19
# baseline (speedup 1.0000x reference)
"""Trainium2 Bass kernel for nn_Head (sparse attention head).

Computation (per batch b):
    K = X @ Wk; Q = X @ Wq; V = X @ Wv                       # [T, HS]
    S = Q K^T / sqrt(HS)                                     # [T, T]
    A = softmax_row(where(dag==0, -inf, S))                  # row-wise over keys
    out[j, h] = sum_i A[i, j] V[i, h]   (transposed AV)      # [T, HS]
    return swish(out)

Sharding over 8 NeuronCores: core = (b, h) with b = batch (4), h = query-row
half (2).  X^T is rolled so the core's query shard occupies columns [0, TH);
one X^T copy serves the K/Q/V projections.

Phase A computes only K^T-half0 and the first Q chunk, so the scores
pipeline (phase B) starts almost immediately.  The remaining projections
(Q chunks 1-3, K^T-half1, all of V) run inside phase B as "corner borrows":
their matmuls target a corner of the upcoming S PSUM tile and are drained
by DVE before the QK matmul resets that region -- zero extra PSUM banks and
no pipeline slot theft.  The mask streams as fp8 (0/1 exact) from t=0 on
its own DMA queue.  Masked softmax numerator U = exp(S/8) * dag via one DVE
scalar_tensor_tensor per (block, j-half) with fused row-sum accumulate; the
1/l normalizer folds into the V stationary in two half-batches as the row
sums complete; phase C accumulates the transposed AV jq-outer in PSUM so
each 512-column slice drains (fp16) and DMAs out while the rest
accumulates.  Host sums the two partials per batch in fp32, divides by
VSCALE, transposes, applies swish.
"""

import sys

for _p in ("/opt/trn_rl_repo",):
    if _p not in sys.path:
        sys.path.append(_p)

import numpy as np

import concourse.bacc as bacc
import concourse.mybir as mybir
import concourse.tile as tile
from concourse.bass_utils import run_bass_kernel_spmd

B, T, D, HS = 4, 4096, 512, 64
TH = T // 2          # query rows per core
P = 128              # partitions
NB = TH // P         # 16 i-blocks per core
NCC = D // P         # 4 contraction chunks over D
NJ = 512             # matmul moving free dim
JH = T // 2          # j-half width for the exp/mask pipeline
VSCALE = 1024.0      # fp16 dynamic-range scale folded into V/l

F16 = mybir.dt.float16
F32 = mybir.dt.float32
F8 = mybir.dt.float8e4
AF = mybir.ActivationFunctionType
ALU = mybir.AluOpType

_CACHE = {}


def _build():
    if "nc" in _CACHE:
        return _CACHE["nc"]

    nc = bacc.Bacc("TRN2", target_bir_lowering=False, debug=False)

    xt_d = nc.dram_tensor("xt", [D, T], F16, kind="ExternalInput").ap()
    m_d = nc.dram_tensor("m", [TH, T], F8, kind="ExternalInput").ap()
    wk_d = nc.dram_tensor("wk", [D, HS], F16, kind="ExternalInput").ap()
    wq_d = nc.dram_tensor("wq", [D, HS], F16, kind="ExternalInput").ap()
    wv_d = nc.dram_tensor("wv", [D, HS], F16, kind="ExternalInput").ap()
    ot_d = nc.dram_tensor("ot", [HS, T], F16, kind="ExternalOutput").ap()

    with tile.TileContext(nc) as tc:
        with (
            tc.tile_pool(name="persist", bufs=1) as pp,
            tc.tile_pool(name="phBm", bufs=5) as pBm,
            tc.tile_pool(name="phB", bufs=3) as pB,
        ):
            kt = pp.tile([HS, T], F16, tag="kt")          # K^T (all keys)
            qt = pp.tile([HS, TH], F16, tag="qt")         # Q^T (shard rows)
            v = pp.tile([P, NB * HS], F16, tag="v")       # V rows (shard)
            vt = pp.tile([P, NB * HS], F16, tag="vt")     # V/l * VSCALE
            lh = pp.tile([P, 2 * NB], F32, tag="lh")      # row-sum halves
            u = pp.tile([P, NB * T], F16, tag="u")        # masked exp(S/8)
            xq = pp.tile([P, NCC * TH], F16, tag="xq")    # X^T shard cols
            xk1 = pp.tile([P, NCC * JH], F16, tag="xk1")  # X^T cols half1
            wv_s = pp.tile([P, NCC * HS], F16, tag="wv")
            wk_s = pp.tile([P, NCC * HS], F16, tag="wk")
            wq_s = pp.tile([P, NCC * HS], F16, tag="wq")

            # ---- phase A: load X^T halves + weights; K-half0, Q chunk 0 ----
            with tc.tile_pool(name="psA", bufs=2, space="PSUM") as psA:
                for ci in range(NCC):
                    cs = slice(ci * P, (ci + 1) * P)
                    nc.scalar.dma_start(wk_s[:, ci * HS:(ci + 1) * HS], wk_d[cs, :])
                    nc.scalar.dma_start(wq_s[:, ci * HS:(ci + 1) * HS], wq_d[cs, :])
                    nc.scalar.dma_start(wv_s[:, ci * HS:(ci + 1) * HS], wv_d[cs, :])
                for ci in range(NCC):
                    cs = slice(ci * P, (ci + 1) * P)
                    eng = nc.sync if ci < 2 else nc.scalar
                    eng.dma_start(xq[:, ci * TH:(ci + 1) * TH], xt_d[cs, 0:JH])
                for ci in range(NCC):
                    cs = slice(ci * P, (ci + 1) * P)
                    nc.scalar.dma_start(
                        xk1[:, ci * JH:(ci + 1) * JH], xt_d[cs, JH:T]
                    )

                # PE warm-up during the DMA head: dummy matmuls on a local
                # constant tile keep the Tensor engine busy (and its DVFS
                # clock ramping) before the first real projection.
                wu = pp.tile([P, NJ], F16, tag="wu")
                nc.vector.memset(wu[:], 0.0)
                for i in range(16):
                    wup = psA.tile([P, NJ], F32, tag="pw")
                    nc.tensor.matmul(
                        wup[:], wu[0:HS, 0:P], wu[0:HS, :],
                        start=True, stop=True,
                    )

                for j0 in range(0, JH, NJ):
                    ktp = psA.tile([HS, NJ], F32, tag="pj")
                    for ci in range(NCC):
                        nc.tensor.matmul(
                            ktp[:],
                            wk_s[:, ci * HS:(ci + 1) * HS],
                            xq[:, ci * TH + j0: ci * TH + j0 + NJ],
                            start=(ci == 0),
                            stop=(ci == NCC - 1),
                        )
                    nc.scalar.copy(kt[:, j0:j0 + NJ], ktp[:])

                qtp = psA.tile([HS, NJ], F32, tag="pj")
                for ci in range(NCC):
                    nc.tensor.matmul(
                        qtp[:],
                        wq_s[:, ci * HS:(ci + 1) * HS],
                        xq[:, ci * TH: ci * TH + NJ],
                        start=(ci == 0),
                        stop=(ci == NCC - 1),
                    )
                nc.scalar.copy(qt[:, 0:NJ], qtp[:])

            # ---- phase B: per (j-half, i-block): scores, exp, mask+rowsum --
            # Corner borrows: remaining projections write into a corner of
            # the upcoming S tile, are drained by DVE, then QK resets it.
            with tc.tile_pool(name="psB", bufs=2, space="PSUM") as psB:
                for jh in range(2):
                    for k in range(NB):
                        it = jh * NB + k
                        mk = pBm.tile([P, JH], F8, tag="mask")
                        nc.sync.dma_start(
                            mk[:], m_d[k * P:(k + 1) * P, jh * JH:(jh + 1) * JH]
                        )
                        sp = psB.tile([P, JH], F32, tag="s")

                        if 1 <= it < 4:
                            # Q chunk `it` (blocks 4it..4it+3)
                            j0 = it * NJ
                            cnr = sp[0:HS, JH - NJ:JH]
                            for ci in range(NCC):
                                nc.tensor.matmul(
                                    cnr,
                                    wq_s[:, ci * HS:(ci + 1) * HS],
                                    xq[:, ci * TH + j0: ci * TH + j0 + NJ],
                                    start=(ci == 0),
                                    stop=(ci == NCC - 1),
                                )
                            nc.vector.tensor_copy(qt[:, j0:j0 + NJ], cnr)
                        elif 4 <= it < 8:
                            # K^T half-1 chunk it-4
                            j0 = (it - 4) * NJ
                            cnr = sp[0:HS, JH - NJ:JH]
                            for ci in range(NCC):
                                nc.tensor.matmul(
                                    cnr,
                                    wk_s[:, ci * HS:(ci + 1) * HS],
                                    xk1[:, ci * JH + j0: ci * JH + j0 + NJ],
                                    start=(ci == 0),
                                    stop=(ci == NCC - 1),
                                )
                            nc.vector.tensor_copy(kt[:, JH + j0:JH + j0 + NJ], cnr)
                        elif 8 <= it < 24:
                            # V block it-8
                            kb = it - 8
                            cnr = sp[:, JH - HS:JH]
                            for ci in range(NCC):
                                nc.tensor.matmul(
                                    cnr,
                                    xq[:, ci * TH + kb * P: ci * TH + (kb + 1) * P],
                                    wv_s[:, ci * HS:(ci + 1) * HS],
                                    start=(ci == 0),
                                    stop=(ci == NCC - 1),
                                )
                            nc.vector.tensor_copy(
                                v[:, kb * HS:(kb + 1) * HS], cnr
                            )

                        for jq in range(JH // NJ):
                            nc.tensor.matmul(
                                sp[:, jq * NJ:(jq + 1) * NJ],
                                qt[:, k * P:(k + 1) * P],
                                kt[:, jh * JH + jq * NJ: jh * JH + (jq + 1) * NJ],
                                start=True,
                                stop=True,
                            )
                        er = pB.tile([P, JH], F16, tag="eraw")
                        nc.scalar.activation(er[:], sp[:], AF.Exp, scale=0.125)
                        nc.vector.scalar_tensor_tensor(
                            out=u[:, k * T + jh * JH: k * T + (jh + 1) * JH],
                            in0=er[:],
                            scalar=1.0,
                            in1=mk[:],
                            op0=ALU.mult,
                            op1=ALU.mult,
                            accum_out=lh[:, jh * NB + k: jh * NB + k + 1],
                        )

                        if jh == 1 and (k == NB // 2 - 1 or k == NB - 1):
                            # normalizer fold for the half whose row-sums just
                            # completed: l = lh0 + lh1; vt = v * (VSCALE/l)
                            h0 = 0 if k == NB // 2 - 1 else NB // 2
                            lt = pB.tile([P, NB // 2], F32, tag="lt")
                            rl = pB.tile([P, NB // 2], F32, tag="rl")
                            nc.vector.tensor_tensor(
                                out=lt[:],
                                in0=lh[:, h0:h0 + NB // 2],
                                in1=lh[:, NB + h0:NB + h0 + NB // 2],
                                op=ALU.add,
                            )
                            nc.vector.reciprocal(rl[:], lt[:])
                            nc.vector.tensor_scalar(
                                out=lt[:], in0=rl[:], scalar1=VSCALE,
                                scalar2=None, op0=ALU.mult,
                            )
                            for kb in range(h0, h0 + NB // 2):
                                nc.vector.tensor_scalar(
                                    out=vt[:, kb * HS:(kb + 1) * HS],
                                    in0=v[:, kb * HS:(kb + 1) * HS],
                                    scalar1=lt[:, kb - h0:kb - h0 + 1],
                                    scalar2=None,
                                    op0=ALU.mult,
                                )

            # ---- phase C: OT = sum_k vt_k^T . u_k  (transposed AV) ----
            # jq-outer so each 512-column slice finishes early and its
            # fp16 conversion + DMA-out overlap the remaining accumulation.
            with (
                tc.tile_pool(name="psC", bufs=4, space="PSUM") as psC,
                tc.tile_pool(name="phC", bufs=2) as pC,
            ):
                for jq in range(T // NJ):
                    otp = psC.tile([HS, NJ], F32, tag="ot")
                    for k in range(NB):
                        nc.tensor.matmul(
                            otp[:],
                            vt[:, k * HS:(k + 1) * HS],
                            u[:, k * T + jq * NJ: k * T + (jq + 1) * NJ],
                            start=(k == 0),
                            stop=(k == NB - 1),
                        )
                    ot_sb = pC.tile([HS, NJ], F16, tag="ot_sb")
                    nc.scalar.copy(ot_sb[:], otp[:])
                    nc.scalar.dma_start(
                        ot_d[:, jq * NJ:(jq + 1) * NJ], ot_sb[:]
                    )

    nc.compile()
    _CACHE["nc"] = nc
    return nc


def _prep_inputs(X, dag, Wk, Wq, Wv):
    import ml_dtypes

    X = np.asarray(X, dtype=np.float32)
    dag = np.asarray(dag)
    w16 = {
        "wk": np.asarray(Wk, dtype=np.float16),
        "wq": np.asarray(Wq, dtype=np.float16),
        "wv": np.asarray(Wv, dtype=np.float16),
    }
    m8 = (dag != 0).astype(ml_dtypes.float8_e4m3fn)
    in_maps = []
    for core in range(8):
        b, h = divmod(core, 2)
        xb = X[b].astype(np.float16)
        # roll X^T so this core's query shard sits at columns [0, TH); the
        # key order (and mask columns, and output columns) rotate with it.
        xtr = np.roll(xb.T, -h * TH, axis=1)
        mr = np.roll(m8[h * TH:(h + 1) * TH], -h * TH, axis=1)
        in_maps.append(
            {
                "xt": np.ascontiguousarray(xtr),
                "m": np.ascontiguousarray(mr),
                **w16,
            }
        )
    return in_maps


def kernel(X, dag, Wk, Wq, Wv, _trace=False):
    nc = _build()
    in_maps = _prep_inputs(X, dag, Wk, Wq, Wv)
    res = run_bass_kernel_spmd(nc, in_maps, list(range(8)), trace=_trace)
    out = np.empty((B, T, HS), dtype=np.float32)
    for b in range(B):
        o0 = res.results[2 * b]["ot"].astype(np.float32)
        o1 = np.roll(res.results[2 * b + 1]["ot"].astype(np.float32), TH, axis=1)
        o = (o0 + o1).T / np.float32(VSCALE)
        out[b] = o / (1.0 + np.exp(-o))  # swish: o * sigmoid(o)
    if _trace:
        return out, res
    return out
